# revision 11
# baseline (speedup 1.0000x reference)
"""LogitLinear Trainium2 kernel: softmax-moment weights + dual GEMM.

out[n, 0, o] = sum_i mean(W_logits[:, o, i]) * x[n, i]   + mean(b_logits[:, o])
out[n, 1, o] = sum_i var(W_logits[:, o, i])  * x[n, i]^2 + var(b_logits[:, o])

p = softmax(logits over D=3 values [-1, 0, 1]); mean = p2 - p0,
E[w^2] = p0 + p2, var = E[w^2] - mean^2.

Sharding: out_feat split across 8 cores (512 each); x replicated.
Host pre-transposes W (i-major) and x (x^T, bf16) so both GEMM operands
load with the contraction dim on partitions using contiguous DMA.
"""

import numpy as np
import ml_dtypes

N, IN, OUT, D = 2048, 4096, 4096, 3
NCORES = 8
OS = OUT // NCORES  # 512 out-features per core
KB = IN // 128      # 32 contraction blocks
PAIR = 2            # kb processed per moment step
KQ = KB // PAIR
NT = N // 128       # 16 n-tiles
WAVE = 4            # n-tiles per PSUM wave
NWAVES = NT // WAVE
WS = WAVE * 128     # 512 columns per wave
SKEW = 1            # kq-skew for the var-weight chain (m2 on ACT)

_CACHED_NC = None


def _build():
    global _CACHED_NC
    if _CACHED_NC is not None:
        return _CACHED_NC
    import concourse.bass as bass
    import concourse.bacc as bacc
    import concourse.mybir as mybir
    import concourse.tile as tile

    dt = mybir.dt
    f32, bf16 = dt.float32, dt.bfloat16
    Exp = mybir.ActivationFunctionType.Exp
    Square = mybir.ActivationFunctionType.Square

    nc = bacc.Bacc("TRN2", debug=False, num_devices=NCORES)
    xt = nc.dram_tensor("xt", [IN, N], bf16, kind="ExternalInput")
    wt = nc.dram_tensor("wt", [D, IN, OS], f32, kind="ExternalInput")
    bl = nc.dram_tensor("bl", [D, OS], f32, kind="ExternalInput")
    out = nc.dram_tensor("out", [N, 2, OS], f32, kind="ExternalOutput")

    # kb-pair views: partition = i within block, p2 = which kb of the pair
    xt_ap = xt.ap().rearrange("(kq p2 p) n -> kq p p2 n", p=128, p2=PAIR)
    wt_ap = wt.ap().rearrange("d (kq p2 p) o -> kq p d p2 o", p=128, p2=PAIR)
    out_ap = out.ap().rearrange("(nt p) m o -> nt p m o", p=128)

    with tile.TileContext(nc) as tc:
        with (
            tc.tile_pool(name="wres", bufs=1) as wres,
            tc.tile_pool(name="ld", bufs=2) as ld,
            tc.tile_pool(name="mt", bufs=2) as mt,
            tc.tile_pool(name="xs", bufs=4) as xs,
            tc.tile_pool(name="st", bufs=3) as st,
            tc.tile_pool(name="bias", bufs=1) as bias,
            tc.tile_pool(name="ps", bufs=8, space="PSUM") as ps,
        ):
            wTm = wres.tile([128, KB, OS], bf16, tag="wTm")
            wTv = wres.tile([128, KB, OS], bf16, tag="wTv")

            # warm the ACT exp table set before the first real exp needs it
            warm = wres.tile([1, 8], f32, tag="warm")
            nc.vector.memset(warm, 0.0)
            nc.scalar.activation(out=warm, in_=warm, func=Exp)

            s1_t = [None] * KQ
            rb_t = [None] * KQ

            def emit_moments_front(kq):
                lt = ld.tile([128, D, PAIR, OS], f32, tag="lt")
                for di in range(D):
                    nc.sync.dma_start(out=lt[:, di], in_=wt_ap[kq][:, di])
                e = mt.tile([128, D, PAIR, OS], bf16, tag="e")
                nc.scalar.activation(out=e, in_=lt, func=Exp)
                s1 = mt.tile([128, PAIR, OS], bf16, tag="s1", bufs=SKEW + 2)
                nc.gpsimd.tensor_add(s1, e[:, 2], e[:, 0])
                s = mt.tile([128, PAIR, OS], f32, tag="s")
                nc.gpsimd.tensor_add(s, s1, e[:, 1])
                r = mt.tile([128, PAIR, OS], f32, tag="r")
                nc.vector.reciprocal_approx_fast(out=r, in_=s)
                rb = mt.tile([128, PAIR, OS], bf16, tag="rb", bufs=SKEW + 2)
                nc.vector.tensor_copy(rb, r)
                a = mt.tile([128, PAIR, OS], bf16, tag="a")
                nc.vector.tensor_sub(a, e[:, 2], e[:, 0])
                nc.vector.tensor_mul(
                    wTm[:, PAIR * kq : PAIR * (kq + 1), :], a, rb
                )
                s1_t[kq], rb_t[kq] = s1, rb

            def emit_moments_back(kq):
                msl = wTm[:, PAIR * kq : PAIR * (kq + 1), :]
                m2 = mt.tile([128, PAIR, OS], bf16, tag="m2")
                nc.scalar.activation(out=m2, in_=msl, func=Square)
                sq = mt.tile([128, PAIR, OS], bf16, tag="sq")
                nc.gpsimd.tensor_mul(sq, s1_t[kq], rb_t[kq])
                nc.vector.tensor_sub(
                    wTv[:, PAIR * kq : PAIR * (kq + 1), :], sq, m2
                )
                s1_t[kq] = rb_t[kq] = None

            def emit_bias():
                bl_ap = bl.ap()
                bl_bcast = bass.AP(
                    tensor=bl_ap.tensor,
                    offset=bl_ap.offset,
                    ap=[[0, 128]] + [list(p) for p in bl_ap.ap],
                )
                bl_t = ld.tile([128, D, OS], f32, tag="lt")
                nc.gpsimd.dma_start(out=bl_t, in_=bl_bcast)
                eb = mt.tile([128, D, OS], f32, tag="e")
                nc.scalar.activation(out=eb, in_=bl_t, func=Exp)
                bs1 = mt.tile([128, OS], f32, tag="s1", bufs=SKEW + 2)
                nc.vector.tensor_add(bs1, eb[:, 2, :], eb[:, 0, :])
                bs = mt.tile([128, OS], f32, tag="s")
                nc.vector.tensor_add(bs, bs1, eb[:, 1, :])
                br = mt.tile([128, OS], f32, tag="r")
                nc.vector.reciprocal_approx_fast(out=br, in_=bs)
                bA = mt.tile([128, OS], f32, tag="a")
                nc.vector.tensor_sub(bA, eb[:, 2, :], eb[:, 0, :])
                bmean = bias.tile([128, OS], f32, tag="bmean")
                nc.vector.tensor_mul(bmean, bA, br)
                bm2 = mt.tile([128, OS], f32, tag="m2")
                nc.vector.tensor_mul(bm2, bmean, bmean)
                bsq = mt.tile([128, OS], f32, tag="sq")
                nc.vector.tensor_mul(bsq, bs1, br)
                bvar = bias.tile([128, OS], f32, tag="bvar")
                nc.vector.tensor_sub(bvar, bsq, bm2)
                return bmean, bvar

            bmean = bvar = None
            for w in range(NWAVES):
                psm = [
                    ps.tile([128, OS], f32, tag="ps", name=f"psm{w}_{j}")
                    for j in range(WAVE)
                ]
                psv = [
                    ps.tile([128, OS], f32, tag="ps", name=f"psv{w}_{j}")
                    for j in range(WAVE)
                ]
                first = w == 0
                xx_slabs = {}

                def emit_var_mms(kq):
                    for kbi in range(PAIR):
                        kb = PAIR * kq + kbi
                        for j in range(WAVE):
                            nc.tensor.matmul(
                                psv[j],
                                lhsT=xx_slabs[kq][
                                    :, kbi, j * 128 : (j + 1) * 128
                                ],
                                rhs=wTv[:, kb, :],
                                start=(kb == 0),
                                stop=(kb == KB - 1),
                            )
                    del xx_slabs[kq]

                for kq in range(KQ):
                    if first:
                        emit_moments_front(kq)
                        if kq >= SKEW:
                            emit_moments_back(kq - SKEW)
                    xsl = xs.tile([128, PAIR, WS], bf16, tag="xsl")
                    nc.sync.dma_start(
                        out=xsl, in_=xt_ap[kq][:, :, w * WS : (w + 1) * WS]
                    )
                    xxl = xs.tile(
                        [128, PAIR, WS], bf16, tag="xxl", bufs=SKEW + 3
                    )
                    nc.vector.tensor_mul(xxl, xsl, xsl)
                    xx_slabs[kq] = xxl
                    for kbi in range(PAIR):
                        kb = PAIR * kq + kbi
                        for j in range(WAVE):
                            nc.tensor.matmul(
                                psm[j],
                                lhsT=xsl[:, kbi, j * 128 : (j + 1) * 128],
                                rhs=wTm[:, kb, :],
                                start=(kb == 0),
                                stop=(kb == KB - 1),
                            )
                    kqv = kq - SKEW if first else kq
                    if kqv >= 0:
                        emit_var_mms(kqv)
                if first:
                    bmean, bvar = emit_bias()
                    for kq in range(KQ - SKEW, KQ):
                        emit_moments_back(kq)
                        emit_var_mms(kq)
                for j in range(WAVE):
                    stg = st.tile([128, 2, OS], f32, tag="stg")
                    nc.vector.tensor_add(stg[:, 0, :], psm[j], bmean)
                    nc.vector.tensor_add(stg[:, 1, :], psv[j], bvar)
                    nc.sync.dma_start(out=out_ap[w * WAVE + j], in_=stg)

    nc.compile()
    _CACHED_NC = nc
    return nc


def kernel(x, W_logits, b_logits):
    from concourse import bass_utils

    nc = _build()
    xt_b = np.ascontiguousarray(x.T).astype(ml_dtypes.bfloat16)
    in_maps = []
    for c in range(NCORES):
        sl = slice(c * OS, (c + 1) * OS)
        wt_c = np.ascontiguousarray(W_logits[:, sl, :].transpose(0, 2, 1))
        bl_c = np.ascontiguousarray(b_logits[:, sl, 0])
        in_maps.append({"xt": xt_b, "wt": wt_c, "bl": bl_c})
    res = bass_utils.run_bass_kernel_spmd(
        nc, in_maps, core_ids=list(range(NCORES))
    )
    full = np.empty((N, 2, OUT), dtype=np.float32)
    for c in range(NCORES):
        full[:, :, c * OS : (c + 1) * OS] = res.results[c]["out"]
    return full
